# revision 7
# baseline (speedup 1.0000x reference)
# GPT-2 (6L, D=768, S=2048, V=50304) forward pass on 8 trn2 NeuronCores.
#
# Sharding:
#  - Body: sequence-parallel. The 2048 tokens are split into 16 blocks of 128;
#    core c owns blocks (c, 15-c) — "snake" pairing so causal-attention work is
#    balanced across cores. Each core runs the full 6-layer stack for its 256
#    tokens; K/V are exchanged with one AllGather per layer.
#  - lm_head: vocab-parallel. After a final AllGather of the hidden states,
#    core c computes logits[:, c*6288:(c+1)*6288] and the host concatenates.
#
# On-chip layout: activations are kept TRANSPOSED (d_model on partitions,
# tokens on the free axis) in 6 slabs of 128 dims. With natural [in,out]
# weight layouts every matmul in the network maps onto nc.tensor.matmul with
# zero transposes:
#   Q_T/K_T  : lhsT = W[e-chunk, d-chunk],   rhs = h_T[e-chunk]     (out [d, q])
#   V        : lhsT = h_T[e-chunk, t-chunk], rhs = W[e-chunk]       (out [t, d])
#   scores_T : lhsT = K_T[head, kv-chunk],   rhs = Q_T[head]        (out [kv, q])
#   PV       : lhsT = V_aug[kv-chunk, head], rhs = exp_T[kv-chunk]  (out [d+1, q])
#   out-proj : lhsT = Wo[f-chunk, e-chunk],  rhs = attn_T[f-chunk]  (out [e, q])
# Softmax runs over kv (the partition axis of scores_T): max-subtraction is
# skipped (scores are bounded ~±3 for this init) and the denominator comes for
# free from an all-ones column appended to V (row 64 of the PV accumulator).
# Causality/core-assignment is pure data: a host-built per-core mask multiplies
# exp(scores); the program is identical on all 8 cores (SPMD).
#
# Numerics: weights and matmul operands bf16 (host-cast), accumulation fp32
# (PSUM), residual stream / LN stats / softmax fp32. LN statistics use fp32r
# ones-matmuls over the partition axis. Measured end-to-end scale-relative
# absmax error vs the fp32 reference: ~0.7e-2 (bf16-dominated).

import numpy as np
import ml_dtypes

import concourse.bass as bass
import concourse.mybir as mybir
import concourse.tile as tile
from concourse import bacc
from concourse.tile import add_dep_helper

F32 = mybir.dt.float32
F32R = mybir.dt.float32r
BF16 = mybir.dt.bfloat16
BF = ml_dtypes.bfloat16

P = 128
L, D, H, HD, FF, S, V = 6, 768, 12, 64, 3072, 2048, 50304
NCORE = 8
NB = S // P            # 16 token blocks
TPC = 2 * P            # 256 tokens per core
EC = D // P            # 6 d_model slabs
FS = FF // P           # 24 ff slabs
VS = V // NCORE        # 6288 vocab shard
VAUG = H * (HD + 1)    # 780 (V with ones column per head)
EPS = 1e-5
SCALE = 1.0 / float(np.sqrt(HD))

NK = EC * P * TPC          # K_T staging elems per core
NV = 2 * P * VAUG          # V_aug staging elems per core
NZ = EC * P * TPC          # final hidden staging
RG = [list(range(NCORE))]


def _seg_half(b):
    """Global kv-block b -> (AG segment, half) under snake pairing."""
    return (b, 0) if b < 8 else (15 - b, 1)


def _kcol(b):
    r, u = _seg_half(b)
    return r * TPC + u * P


def _vchunk(b):
    r, u = _seg_half(b)
    return 2 * r + u


def _block_of_tchunk(tc):
    r, u = tc // 2, tc % 2
    return r if u == 0 else 15 - r


def build_nc(nl=L):
    nc = bacc.Bacc("TRN2", target_bir_lowering=False, debug=False,
                   num_devices=NCORE)

    # ---- per-core DRAM inputs ----
    xwte = nc.dram_tensor("xwte", [EC, P, TPC], F32, kind="ExternalInput")
    xwpe = nc.dram_tensor("xwpe", [EC, P, TPC], F32, kind="ExternalInput")
    wq = nc.dram_tensor("wq", [nl, D, D], BF16, kind="ExternalInput")
    wk = nc.dram_tensor("wk", [nl, D, D], BF16, kind="ExternalInput")
    wv = nc.dram_tensor("wv", [nl, D, D], BF16, kind="ExternalInput")
    wo = nc.dram_tensor("wo", [nl, D, D], BF16, kind="ExternalInput")
    w1 = nc.dram_tensor("w1", [nl, D, FF], BF16, kind="ExternalInput")
    w2 = nc.dram_tensor("w2", [nl, FF, D], BF16, kind="ExternalInput")
    bqkv = nc.dram_tensor("bqkv", [nl, 4, D], F32, kind="ExternalInput")
    b1d = nc.dram_tensor("b1d", [nl, FF], F32, kind="ExternalInput")
    b2d = nc.dram_tensor("b2d", [nl, D], F32, kind="ExternalInput")
    lng = nc.dram_tensor("lng", [nl, 2, D], F32, kind="ExternalInput")
    lnb = nc.dram_tensor("lnb", [nl, 2, D], F32, kind="ExternalInput")
    lnf = nc.dram_tensor("lnf", [2, D], F32, kind="ExternalInput")
    wlm = nc.dram_tensor("wlm", [D, VS], BF16, kind="ExternalInput")
    mskd = nc.dram_tensor("mskd", [P, NB, TPC], BF16, kind="ExternalInput")
    out = nc.dram_tensor("logits", [S, VS], F32, kind="ExternalOutput")

    Add = mybir.AluOpType.add
    Sub = mybir.AluOpType.subtract
    Mult = mybir.AluOpType.mult
    AF = mybir.ActivationFunctionType

    from contextlib import ExitStack
    with tile.TileContext(nc, num_cores=NCORE) as tc, ExitStack() as ctx:
        sing = ctx.enter_context(tc.tile_pool(name="sing", bufs=1))
        wp = ctx.enter_context(tc.tile_pool(name="wp", bufs=1))
        work = ctx.enter_context(tc.tile_pool(name="work", bufs=2))
        pp = ctx.enter_context(tc.tile_pool(name="pp", bufs=1, space="PSUM"))
        dram = ctx.enter_context(tc.tile_pool(name="dram", bufs=1,
                                              space="DRAM"))

        # ---- persistent SBUF ----
        xT = sing.tile([P, EC, TPC], F32)       # residual stream, transposed
        hT = sing.tile([P, EC, TPC], BF16)      # LN output
        gT = sing.tile([P, FS, TPC], BF16)      # gelu(fc1) output
        qT = sing.tile([P, EC, TPC], BF16)
        kTl = sing.tile([P, EC, TPC], BF16)     # local K_T (staged to AG)
        vaug = sing.tile([P, 2, VAUG], BF16)    # local V+ones (staged to AG)
        xaT = sing.tile([P, EC, TPC], BF16)     # attention output, transposed
        msk = sing.tile([P, NB, TPC], BF16)
        ones = sing.tile([P, 1], BF16)
        eps_t = sing.tile([1, 1], F32)
        vbb = sing.tile([P, D], F32)            # v-bias broadcast
        vbrow = sing.tile([1, D], F32)
        bq_sb = sing.tile([P, nl, 4, EC], F32)
        b1_sb = sing.tile([P, nl, FS], F32)
        b2_sb = sing.tile([P, nl, EC], F32)
        lng_sb = sing.tile([P, nl, 2, EC], F32)
        lnb_sb = sing.tile([P, nl, 2, EC], F32)
        lnf_sb = sing.tile([P, 2, EC], F32)

        nc.vector.memset(ones, 1.0)
        nc.vector.memset(eps_t, EPS)
        nc.sync.dma_start(msk, mskd.ap())
        nc.sync.dma_start(
            bq_sb, bqkv.ap().rearrange("l f (ec p) -> p l f ec", p=P))
        nc.sync.dma_start(
            b1_sb, b1d.ap().rearrange("l (fs p) -> p l fs", p=P))
        nc.sync.dma_start(
            b2_sb, b2d.ap().rearrange("l (ec p) -> p l ec", p=P))
        nc.sync.dma_start(
            lng_sb, lng.ap().rearrange("l i (ec p) -> p l i ec", p=P))
        nc.sync.dma_start(
            lnb_sb, lnb.ap().rearrange("l i (ec p) -> p l i ec", p=P))
        nc.sync.dma_start(
            lnf_sb, lnf.ap().rearrange("i (ec p) -> p i ec", p=P))

        # ---- embeddings: xT = wte_rows_T + wpe_rows_T ----
        nc.sync.dma_start(xT, xwte.ap().transpose([1, 0, 2]))
        for e in range(EC):
            we = work.tile([P, TPC], F32, tag="sq")
            nc.sync.dma_start(we, xwpe.ap()[e])
            nc.vector.tensor_tensor(xT[:, e, :], xT[:, e, :], we, Add)

        def mm_chain(insts, out_ap, lhsT, rhs, start, stop):
            i = nc.tensor.matmul(out_ap, lhsT, rhs, start=start, stop=stop,
                                 skip_group_check=True)
            if insts:
                add_dep_helper(i.ins, insts[-1].ins, sync=False,
                               reason="psum bank write order")
            insts.append(i)
            return i

        def layer_norm(g_of, b_of, dst):
            """LN over d_model (partition axis, 6 slabs) of xT -> dst (bf16).

            g_of(e)/b_of(e) give per-partition [P,1] gamma/beta APs."""
            s1 = pp.tile([1, TPC], F32, tag="accA", bufs=4, name="s1")
            s2 = pp.tile([1, TPC], F32, tag="accA", bufs=4, name="s2")
            c1 = []
            c2 = []
            for e in range(EC):
                xb = work.tile([P, TPC], BF16, tag="xb")
                nc.vector.tensor_copy(xb, xT[:, e, :])
                mm_chain(c1, s1, ones, xb, e == 0, e == EC - 1)
                sq = work.tile([P, TPC], BF16, tag="sq")
                nc.vector.tensor_tensor(sq, xT[:, e, :], xT[:, e, :], Mult)
                mm_chain(c2, s2, ones, sq, e == 0, e == EC - 1)
            ta = work.tile([1, TPC], F32, tag="lnsc")  # m -> c
            tb = work.tile([1, TPC], F32, tag="lnsc")  # msq -> var -> sd -> a
            nc.vector.tensor_scalar(ta, s1, 1.0 / D, None, Mult)
            nc.vector.tensor_tensor(tb, ta, ta, Mult)
            nc.vector.scalar_tensor_tensor(tb, s2, 1.0 / D, tb, Mult, Sub)
            nc.scalar.activation(tb, tb, AF.Sqrt, bias=eps_t)
            nc.vector.reciprocal(tb, tb)
            nc.vector.scalar_tensor_tensor(ta, ta, -1.0, tb, Mult, Mult)
            ab = work.tile([P, TPC], F32, tag="lnbc")
            nc.gpsimd.partition_broadcast(ab, tb)
            cb = work.tile([P, TPC], F32, tag="lnbc")
            nc.gpsimd.partition_broadcast(cb, ta)
            for e in range(EC):
                t1 = work.tile([P, TPC], F32, tag="lnt")
                nc.vector.tensor_tensor(t1, xT[:, e, :], ab, Mult)
                t2 = work.tile([P, TPC], F32, tag="lnt")
                nc.vector.tensor_tensor(t2, t1, cb, Add)
                nc.vector.scalar_tensor_tensor(
                    dst[:, e, :], t2, g_of(e),
                    b_of(e).to_broadcast([P, TPC]), Mult, Add)

        # ================= transformer layers =================
        for l in range(nl):
            # ---- LN1 ----
            layer_norm(lambda e: lng_sb[:, l, 0, e:e + 1],
                       lambda e: lnb_sb[:, l, 0, e:e + 1], hT)

            # ---- load Q/K projection weights ----
            wq_t = [wp.tile([P, D], BF16, tag="wproj", bufs=12,
                            name=f"wq{l}_{e}") for e in range(EC)]
            wk_t = [wp.tile([P, D], BF16, tag="wproj", bufs=12,
                            name=f"wk{l}_{e}") for e in range(EC)]
            for e in range(EC):
                nc.sync.dma_start(wq_t[e], wq.ap()[l, e * P:(e + 1) * P, :])
                nc.sync.dma_start(wk_t[e], wk.ap()[l, e * P:(e + 1) * P, :])

            # ---- Q_T, K_T ----
            for dsl in range(EC):
                psq = pp.tile([P, TPC], F32, tag="accA", bufs=4, name="psq")
                c = []
                for e in range(EC):
                    mm_chain(c, psq, wq_t[e][:, dsl * P:(dsl + 1) * P],
                             hT[:, e, :], e == 0, e == EC - 1)
                nc.vector.tensor_scalar(
                    qT[:, dsl, :], psq, bq_sb[:, l, 0, dsl:dsl + 1], None, Add)
            for dsl in range(EC):
                psk = pp.tile([P, TPC], F32, tag="accA", bufs=4, name="psk")
                c = []
                for e in range(EC):
                    mm_chain(c, psk, wk_t[e][:, dsl * P:(dsl + 1) * P],
                             hT[:, e, :], e == 0, e == EC - 1)
                nc.vector.tensor_scalar(
                    kTl[:, dsl, :], psk, bq_sb[:, l, 1, dsl:dsl + 1], None, Add)

            # ---- V (normal layout, with ones columns) ----
            wv_t = [wp.tile([P, D], BF16, tag="wproj", bufs=12,
                            name=f"wv{l}_{e}") for e in range(EC)]
            for e in range(EC):
                nc.sync.dma_start(wv_t[e], wv.ap()[l, e * P:(e + 1) * P, :])
            nc.sync.dma_start(vbrow, bqkv.ap()[l:l + 1, 2, :])
            nc.gpsimd.partition_broadcast(vbb, vbrow)
            for w in range(2):
                for dh in range(2):
                    psv = pp.tile([P, 384], F32, tag="accA", bufs=4,
                                  name="psv")
                    c = []
                    for e in range(EC):
                        mm_chain(c, psv, hT[:, e, w * P:(w + 1) * P],
                                 wv_t[e][:, dh * 384:(dh + 1) * 384],
                                 e == 0, e == EC - 1)
                    dst = vaug[:, w, :].rearrange(
                        "p (h x) -> p h x", h=H)[:, dh * 6:(dh + 1) * 6, 0:HD]
                    nc.vector.tensor_tensor(
                        dst, psv.rearrange("p (h x) -> p h x", h=6),
                        vbb[:, dh * 384:(dh + 1) * 384].rearrange(
                            "p (h x) -> p h x", h=6), Add)
                nc.vector.memset(
                    vaug[:, w, :].rearrange("p (h x) -> p h x", h=H)[:, :, HD:],
                    1.0)

            # ---- AllGather K_T and V_aug ----
            agin = dram.tile([NK + NV], BF16, tag="agin", bufs=2)
            agout = dram.tile([NCORE * (NK + NV)], BF16, tag="agout", bufs=2,
                              addr_space="Shared")
            nc.sync.dma_start(
                agin[0:NK].rearrange("(ec p q) -> p ec q", p=P, q=TPC), kTl)
            nc.sync.dma_start(
                agin[NK:].rearrange("(u p x) -> p u x", p=P, x=VAUG), vaug)
            nc.gpsimd.collective_compute(
                "AllGather", mybir.AluOpType.bypass, replica_groups=RG,
                ins=[agin.opt()], outs=[agout.opt()])

            ktf = wp.tile([P, EC, S], BF16, tag="ktf", bufs=1)
            vfull = wp.tile([P, NB, VAUG], BF16, tag="vfull", bufs=1)
            for r in range(NCORE):
                seg = agout[r * (NK + NV):(r + 1) * (NK + NV)]
                segk = seg[0:NK].rearrange("(ec p q) -> ec p q", p=P, q=TPC)
                for e in range(EC):
                    nc.sync.dma_start(ktf[:, e, r * TPC:(r + 1) * TPC],
                                      segk[e])
                segv = seg[NK:].rearrange("(u p x) -> u p x", p=P, x=VAUG)
                for u in range(2):
                    nc.sync.dma_start(vfull[:, 2 * r + u, :], segv[u])

            # prefetch out-proj + fc1 weights during attention
            wo_t = [wp.tile([P, D], BF16, tag="wproj", bufs=12,
                            name=f"wo{l}_{e}") for e in range(EC)]
            for e in range(EC):
                nc.sync.dma_start(wo_t[e], wo.ap()[l, e * P:(e + 1) * P, :])
            w1_t = [wp.tile([P, FF], BF16, tag="w1", bufs=6,
                            name=f"w1{l}_{e}") for e in range(EC)]
            for e in range(EC):
                nc.sync.dma_start(w1_t[e], w1.ap()[l, e * P:(e + 1) * P, :])

            # ---- attention ----
            # sigma groups: (start, count, qlo) — qlo=0 spans both q-blocks
            groups = [(0, 2, 0), (2, 2, 0), (4, 2, 0), (6, 2, 0),
                      (8, 4, P), (12, 4, P)]
            for p_ in range(EC):  # head pairs
                pv = [pp.tile([HD + 1, TPC], F32, tag="accA", bufs=4,
                              name=f"pv{p_}_{ph}") for ph in range(2)]
                pvc = [[], []]
                for (s0, cnt, qlo) in groups:
                    qw = TPC - qlo
                    sc = [pp.tile([P, 512], F32, tag="accA", bufs=4,
                                  name=f"sc{p_}_{ph}") for ph in range(2)]
                    scc = [[], []]
                    for ph in range(2):
                        rows = slice(ph * HD, (ph + 1) * HD)
                        for i in range(cnt):
                            sg = s0 + i
                            mm_chain(scc[ph], sc[ph][:, i * qw:(i + 1) * qw],
                                     ktf[rows, p_, _kcol(sg):_kcol(sg) + P],
                                     qT[rows, p_, qlo:], i == 0, i == cnt - 1)
                    for ph in range(2):
                        ex = work.tile([P, 512], BF16, tag="expT", bufs=4)
                        nc.scalar.activation(ex, sc[ph], AF.Exp, scale=SCALE)
                        nc.vector.tensor_tensor(
                            ex.rearrange("p (i q) -> p i q",
                                         i=512 // qw)[:, 0:cnt, :],
                            ex.rearrange("p (i q) -> p i q",
                                         i=512 // qw)[:, 0:cnt, :],
                            msk[:, s0:s0 + cnt, qlo:], Mult)
                        h_ = 2 * p_ + ph
                        for i in range(cnt):
                            sg = s0 + i
                            mm_chain(pvc[ph],
                                     pv[ph][0:HD + 1, qlo:],
                                     vfull[:, _vchunk(sg),
                                           h_ * (HD + 1):(h_ + 1) * (HD + 1)],
                                     ex[:, i * qw:(i + 1) * qw],
                                     sg == 0, sg == NB - 1)
                # normalize + write attention output (transposed layout)
                for ph in range(2):
                    h_ = 2 * p_ + ph
                    rs = work.tile([1, TPC], F32, tag="rs")
                    nc.vector.tensor_copy(rs, pv[ph][HD:HD + 1, :])
                    nc.vector.reciprocal(rs, rs)
                    rb = work.tile([HD, TPC], F32, tag="rb")
                    nc.gpsimd.partition_broadcast(rb, rs)
                    nc.vector.tensor_tensor(
                        xaT[ph * HD:(ph + 1) * HD, p_, :],
                        pv[ph][0:HD, :], rb, Mult)

            # ---- out projection + residual ----
            for e in range(EC):
                pso = pp.tile([P, TPC], F32, tag="accA", bufs=4, name="pso")
                c = []
                for f in range(EC):
                    mm_chain(c, pso, wo_t[f][:, e * P:(e + 1) * P],
                             xaT[:, f, :], f == 0, f == EC - 1)
                nc.vector.scalar_tensor_tensor(
                    xT[:, e, :], pso, bq_sb[:, l, 3, e:e + 1], xT[:, e, :],
                    Add, Add)

            # ---- LN2 + MLP ----
            layer_norm(lambda e: lng_sb[:, l, 1, e:e + 1],
                       lambda e: lnb_sb[:, l, 1, e:e + 1], hT)
            for f in range(FS):
                ps1 = pp.tile([P, TPC], F32, tag="accA", bufs=4, name="ps1")
                c = []
                for e in range(EC):
                    mm_chain(c, ps1, w1_t[e][:, f * P:(f + 1) * P],
                             hT[:, e, :], e == 0, e == EC - 1)
                nc.scalar.activation(gT[:, f, :], ps1, AF.Gelu,
                                     bias=b1_sb[:, l, f:f + 1])
            # fc2: f-outer so w2 tiles stream; one 3-bank accumulator
            ps2 = pp.tile([P, EC, TPC], F32, tag="accB", bufs=1, name="ps2")
            c2ch = []
            for f in range(FS):
                w2t = wp.tile([P, D], BF16, tag="w2", bufs=4,
                              name=f"w2{l}_{f}")
                nc.sync.dma_start(w2t, w2.ap()[l, f * P:(f + 1) * P, :])
                for e in range(EC):
                    mm_chain(c2ch, ps2[:, e, :],
                             w2t[:, e * P:(e + 1) * P], gT[:, f, :],
                             f == 0 and e % 2 == 0, f == FS - 1)
            for e in range(EC):
                nc.vector.scalar_tensor_tensor(
                    xT[:, e, :], ps2[:, e, :], b2_sb[:, l, e:e + 1],
                    xT[:, e, :], Add, Add)

        # ================= final LN + lm_head =================
        layer_norm(lambda e: lnf_sb[:, 0, e:e + 1],
                   lambda e: lnf_sb[:, 1, e:e + 1], hT)

        zin = dram.tile([NZ], BF16, tag="agin", bufs=2)
        zout = dram.tile([NCORE * NZ], BF16, tag="agout", bufs=2,
                         addr_space="Shared")
        nc.sync.dma_start(
            zin.rearrange("(ec p q) -> p ec q", p=P, q=TPC), hT)
        nc.gpsimd.collective_compute(
            "AllGather", mybir.AluOpType.bypass, replica_groups=RG,
            ins=[zin.opt()], outs=[zout.opt()])
        zfull = wp.tile([P, EC, S], BF16, tag="ktf", bufs=1)
        for r in range(NCORE):
            segz = zout[r * NZ:(r + 1) * NZ].rearrange(
                "(ec p q) -> ec p q", p=P, q=TPC)
            for e in range(EC):
                nc.sync.dma_start(zfull[:, e, r * TPC:(r + 1) * TPC], segz[e])

        vchunks = [(i * 512, min(512, VS - i * 512))
                   for i in range((VS + 511) // 512)]
        for (v0, vw) in vchunks:
            wlm_t = wp.tile([P, EC, 512], BF16, tag="wlm", bufs=2,
                            name="wlmt")
            for e in range(EC):
                nc.sync.dma_start(wlm_t[:, e, 0:vw],
                                  wlm.ap()[e * P:(e + 1) * P, v0:v0 + vw])
            for t in range(NB):
                pslm = pp.tile([P, 512], F32, tag="accA", bufs=4, name="pslm")
                c = []
                for e in range(EC):
                    mm_chain(c, pslm[:, 0:vw],
                             zfull[:, e, t * P:(t + 1) * P],
                             wlm_t[:, e, 0:vw], e == 0, e == EC - 1)
                ob = work.tile([P, 512], F32, tag="lmout", bufs=2)
                nc.any.tensor_copy(ob[:, 0:vw], pslm[:, 0:vw])
                b = _block_of_tchunk(t)
                nc.sync.dma_start(out.ap()[b * P:(b + 1) * P, v0:v0 + vw],
                                  ob[:, 0:vw])

    nc.compile()
    return nc


# ======================= host side =======================

def shard_inputs(input_ids, params, nl=L):
    """Build the 8 per-core input maps from the full model inputs."""
    ids = np.asarray(input_ids).reshape(-1)
    p = params
    blocks = p['blocks']
    f32 = np.float32

    def _np(x):
        return np.asarray(x)

    com = {
        'wq': _np(blocks['q_w'])[:nl].astype(BF),
        'wk': _np(blocks['k_w'])[:nl].astype(BF),
        'wv': _np(blocks['v_w'])[:nl].astype(BF),
        'wo': _np(blocks['o_w'])[:nl].astype(BF),
        'w1': _np(blocks['fc1_w'])[:nl].astype(BF),
        'w2': _np(blocks['fc2_w'])[:nl].astype(BF),
        'bqkv': np.stack([_np(blocks['q_b']), _np(blocks['k_b']),
                          _np(blocks['v_b']), _np(blocks['o_b'])],
                         axis=1)[:nl].astype(f32),
        'b1d': _np(blocks['fc1_b'])[:nl].astype(f32),
        'b2d': _np(blocks['fc2_b'])[:nl].astype(f32),
        'lng': np.stack([_np(blocks['ln1_g']), _np(blocks['ln2_g'])],
                        axis=1)[:nl].astype(f32),
        'lnb': np.stack([_np(blocks['ln1_b']), _np(blocks['ln2_b'])],
                        axis=1)[:nl].astype(f32),
        'lnf': np.stack([_np(p['ln_f_g']), _np(p['ln_f_b'])]).astype(f32),
    }
    wte = _np(p['wte'])
    wpe = _np(p['wpe'])
    lm = _np(p['lm_head'])

    in_maps = []
    for c in range(NCORE):
        b0, b1 = c, 15 - c
        tok = np.concatenate([ids[b0 * P:(b0 + 1) * P],
                              ids[b1 * P:(b1 + 1) * P]])
        pos = np.concatenate([np.arange(b0 * P, (b0 + 1) * P),
                              np.arange(b1 * P, (b1 + 1) * P)])
        xw = np.ascontiguousarray(
            wte[tok].T.reshape(EC, P, TPC)).astype(f32)
        xp = np.ascontiguousarray(
            wpe[pos].T.reshape(EC, P, TPC)).astype(f32)
        # mask[k, sigma, w*128+i] for scores_T: keep iff global_k <= global_q
        m = np.zeros((P, NB, TPC), np.float32)
        tri = np.triu(np.ones((P, P), np.float32))  # [k, q]: keep q >= k
        for sg in range(NB):
            for w, bq in ((0, b0), (1, b1)):
                if sg < bq:
                    m[:, sg, w * P:(w + 1) * P] = 1.0
                elif sg == bq:
                    m[:, sg, w * P:(w + 1) * P] = tri
        in_maps.append({
            'xwte': xw, 'xwpe': xp,
            'mskd': m.astype(BF),
            'wlm': np.ascontiguousarray(lm[:, c * VS:(c + 1) * VS]).astype(BF),
            **com,
        })
    return in_maps


def assemble_output(results):
    parts = [np.asarray(r['logits']) for r in results]
    return np.concatenate(parts, axis=1)[None].astype(np.float32)


_NC_CACHE = {}


def get_nc(nl=L):
    if nl not in _NC_CACHE:
        _NC_CACHE[nl] = build_nc(nl)
    return _NC_CACHE[nl]


def kernel(input_ids, params, _trace=False):
    from concourse.bass_utils import run_bass_kernel_spmd
    nc = get_nc()
    in_maps = shard_inputs(input_ids, params)
    res = run_bass_kernel_spmd(nc, in_maps, core_ids=list(range(NCORE)),
                               trace=_trace)
    out = assemble_output(res.results)
    if _trace:
        return out, res
    return out


# revision 20
# speedup vs baseline: 1.2054x; 1.2054x over previous
# GPT-2 (6L, D=768, S=2048, V=50304) forward pass on 8 trn2 NeuronCores.
#
# Sharding:
#  - Body: sequence-parallel. The 2048 tokens are split into 16 blocks of 128;
#    core c owns blocks (c, 15-c) — "snake" pairing so causal-attention work is
#    balanced across cores. Each core runs the full 6-layer stack for its 256
#    tokens; K/V are exchanged with one AllGather per layer.
#  - lm_head: vocab-parallel. After a final AllGather of the hidden states,
#    core c computes logits[:, c*6288:(c+1)*6288] and the host concatenates.
#
# On-chip layout: activations are kept TRANSPOSED (d_model on partitions,
# tokens on the free axis) in 6 slabs of 128 dims. With natural [in,out]
# weight layouts every matmul in the network maps onto nc.tensor.matmul with
# zero transposes:
#   Q_T/K_T  : lhsT = W[e-chunk, d-chunk],   rhs = h_T[e-chunk]     (out [d, q])
#   V        : lhsT = h_T[e-chunk, t-chunk], rhs = W[e-chunk]       (out [t, d])
#   scores_T : lhsT = K_T[head, kv-chunk],   rhs = Q_T[head]        (out [kv, q])
#   PV       : lhsT = V_aug[kv-chunk, head], rhs = exp_T[kv-chunk]  (out [d+1, q])
#   out-proj : lhsT = Wo[f-chunk, e-chunk],  rhs = attn_T[f-chunk]  (out [e, q])
# Softmax runs over kv (the partition axis of scores_T): max-subtraction is
# skipped (scores are bounded ~±3 for this init) and the denominator comes for
# free from an all-ones column appended to V (row 64 of the PV accumulator).
# Causality/core-assignment is pure data: a host-built per-core mask multiplies
# exp(scores); the program is identical on all 8 cores (SPMD).
#
# Numerics: weights and matmul operands bf16 (host-cast), accumulation fp32
# (PSUM), residual stream / LN stats / softmax fp32. LN statistics use fp32r
# ones-matmuls over the partition axis. Measured end-to-end scale-relative
# absmax error vs the fp32 reference: ~0.7e-2 (bf16-dominated).

import numpy as np
import ml_dtypes

import concourse.bass as bass
import concourse.mybir as mybir
import concourse.tile as tile
from concourse import bacc
from concourse.tile import add_dep_helper

F32 = mybir.dt.float32
F32R = mybir.dt.float32r
BF16 = mybir.dt.bfloat16
BF = ml_dtypes.bfloat16

P = 128
L, D, H, HD, FF, S, V = 6, 768, 12, 64, 3072, 2048, 50304
NCORE = 8
NB = S // P            # 16 token blocks
TPC = 2 * P            # 256 tokens per core
EC = D // P            # 6 d_model slabs
FS = FF // P           # 24 ff slabs
VS = V // NCORE        # 6288 vocab shard
VAUG = H * (HD + 1)    # 780 (V with ones column per head)
EPS = 1e-5
SCALE = 1.0 / float(np.sqrt(HD))

NK = EC * P * TPC          # K_T staging elems per core
NV = 2 * P * VAUG          # V_aug staging elems per core
NZ = EC * P * TPC          # final hidden staging
RG = [list(range(NCORE))]


def _seg_half(b):
    """Global kv-block b -> (AG segment, half) under snake pairing."""
    return (b, 0) if b < 8 else (15 - b, 1)


def _kcol(b):
    r, u = _seg_half(b)
    return r * TPC + u * P


def _vchunk(b):
    r, u = _seg_half(b)
    return 2 * r + u


def _block_of_tchunk(tc):
    r, u = tc // 2, tc % 2
    return r if u == 0 else 15 - r


def build_nc(nl=L):
    nc = bacc.Bacc("TRN2", target_bir_lowering=False, debug=False,
                   num_devices=NCORE)

    # ---- per-core DRAM inputs ----
    xwte = nc.dram_tensor("xwte", [EC, P, TPC], F32, kind="ExternalInput")
    xwpe = nc.dram_tensor("xwpe", [EC, P, TPC], F32, kind="ExternalInput")
    wq = nc.dram_tensor("wq", [nl, D, D], BF16, kind="ExternalInput")
    wk = nc.dram_tensor("wk", [nl, D, D], BF16, kind="ExternalInput")
    wv = nc.dram_tensor("wv", [nl, D, D], BF16, kind="ExternalInput")
    wo = nc.dram_tensor("wo", [nl, D, D], BF16, kind="ExternalInput")
    w1 = nc.dram_tensor("w1", [nl, D, FF], BF16, kind="ExternalInput")
    w2 = nc.dram_tensor("w2", [nl, FF, D], BF16, kind="ExternalInput")
    # bias / LN params arrive pre-transposed partition-major from the host
    bqkv = nc.dram_tensor("bqkv", [P, nl, 4, EC], F32, kind="ExternalInput")
    b1d = nc.dram_tensor("b1d", [P, nl, FS], F32, kind="ExternalInput")
    b2d = nc.dram_tensor("b2d", [P, nl, EC], F32, kind="ExternalInput")
    lng = nc.dram_tensor("lng", [P, nl, 2, EC], F32, kind="ExternalInput")
    lnb = nc.dram_tensor("lnb", [P, nl, 2, EC], F32, kind="ExternalInput")
    lnf = nc.dram_tensor("lnf", [P, 2, EC], F32, kind="ExternalInput")
    vbr = nc.dram_tensor("vbr", [nl, D], F32, kind="ExternalInput")
    wlm = nc.dram_tensor("wlm", [D, VS], BF16, kind="ExternalInput")
    mskd = nc.dram_tensor("mskd", [P, NB, TPC], BF16, kind="ExternalInput")
    out = nc.dram_tensor("logits", [S, VS], F32, kind="ExternalOutput")

    Add = mybir.AluOpType.add
    Sub = mybir.AluOpType.subtract
    Mult = mybir.AluOpType.mult
    AF = mybir.ActivationFunctionType

    from contextlib import ExitStack
    with tile.TileContext(nc, num_cores=NCORE) as tc, ExitStack() as ctx:
        sing = ctx.enter_context(tc.tile_pool(name="sing", bufs=1))
        wp = ctx.enter_context(tc.tile_pool(name="wp", bufs=1))
        work = ctx.enter_context(tc.tile_pool(name="work", bufs=2))
        pp = ctx.enter_context(tc.tile_pool(name="pp", bufs=1, space="PSUM"))
        dram = ctx.enter_context(tc.tile_pool(name="dram", bufs=1,
                                              space="DRAM"))

        # ---- persistent SBUF ----
        xT = sing.tile([P, EC, TPC], F32)       # residual stream, transposed
        hT = sing.tile([P, EC, TPC], BF16)      # LN output
        gT = sing.tile([P, FS, TPC], BF16)      # gelu(fc1) output
        qT = sing.tile([P, EC, TPC], BF16)
        kTl = sing.tile([P, EC, TPC], BF16)     # local K_T (staged to AG)
        vaug = sing.tile([P, 2, VAUG], BF16)    # local V+ones (staged to AG)
        xaT = sing.tile([P, EC, TPC], BF16)     # attention output, transposed
        msk = sing.tile([P, NB, TPC], BF16)
        ones = sing.tile([P, 1], BF16)
        eps_t = sing.tile([1, 1], F32)
        vbb = sing.tile([P, D], F32)            # v-bias broadcast
        vbrow = sing.tile([1, D], F32)
        bq_sb = sing.tile([P, nl, 4, EC], F32)
        b1_sb = sing.tile([P, nl, FS], F32)
        b2_sb = sing.tile([P, nl, EC], F32)
        lng_sb = sing.tile([P, nl, 2, EC], F32)
        lnb_sb = sing.tile([P, nl, 2, EC], F32)
        lnf_sb = sing.tile([P, 2, EC], F32)

        nc.vector.memset(ones, 1.0)
        nc.vector.memset(eps_t, EPS)
        nc.sync.dma_start(msk, mskd.ap())
        nc.sync.dma_start(bq_sb, bqkv.ap())
        nc.sync.dma_start(b1_sb, b1d.ap())
        nc.sync.dma_start(b2_sb, b2d.ap())
        nc.sync.dma_start(lng_sb, lng.ap())
        nc.sync.dma_start(lnb_sb, lnb.ap())
        nc.sync.dma_start(lnf_sb, lnf.ap())

        # ---- embeddings: xT = wte_rows_T + wpe_rows_T ----
        nc.sync.dma_start(xT, xwte.ap().transpose([1, 0, 2]))
        for e in range(EC):
            we = work.tile([P, TPC], F32, tag="sq")
            nc.sync.dma_start(we, xwpe.ap()[e])
            nc.vector.tensor_tensor(xT[:, e, :], xT[:, e, :], we, Add)

        def mm_chain(insts, out_ap, lhsT, rhs, start, stop):
            i = nc.tensor.matmul(out_ap, lhsT, rhs, start=start, stop=stop,
                                 skip_group_check=True)
            if insts:
                add_dep_helper(i.ins, insts[-1].ins, sync=False,
                               reason="psum bank write order")
            insts.append(i)
            return i

        def layer_norm(g_of, b_of, dst):
            """LN over d_model (partition axis, 6 slabs) of xT -> dst (bf16).

            g_of(e)/b_of(e) give per-partition [P,1] gamma/beta APs."""
            st = pp.tile([1, 2 * TPC], F32, tag="accA", bufs=4, name="st")
            c1 = []
            for e in range(EC):
                xb = work.tile([P, 2 * TPC], BF16, tag="xb")
                nc.vector.tensor_copy(xb[:, 0:TPC], xT[:, e, :])
                nc.vector.tensor_tensor(xb[:, TPC:], xT[:, e, :],
                                        xT[:, e, :], Mult)
                mm_chain(c1, st, ones, xb, e == 0, e == EC - 1)
            s1 = st[:, 0:TPC]
            s2 = st[:, TPC:]
            ta = work.tile([1, TPC], F32, tag="lnsc")  # m -> c
            tb = work.tile([1, TPC], F32, tag="lnsc")  # msq -> var -> sd -> a
            nc.vector.tensor_scalar(ta, s1, 1.0 / D, None, Mult)
            nc.vector.tensor_tensor(tb, ta, ta, Mult)
            nc.vector.scalar_tensor_tensor(tb, s2, 1.0 / D, tb, Mult, Sub)
            nc.scalar.activation(tb, tb, AF.Sqrt, bias=eps_t)
            nc.vector.reciprocal_approx_fast(tb, tb)
            nc.vector.scalar_tensor_tensor(ta, ta, -1.0, tb, Mult, Mult)
            ab = work.tile([P, TPC], F32, tag="lnbc")
            nc.gpsimd.partition_broadcast(ab, tb)
            cb = work.tile([P, TPC], F32, tag="lnbc")
            nc.gpsimd.partition_broadcast(cb, ta)
            for e in range(EC):
                t1 = work.tile([P, TPC], F32, tag="lnt")
                nc.vector.tensor_tensor(t1, xT[:, e, :], ab, Mult)
                t2 = work.tile([P, TPC], F32, tag="lnt")
                nc.vector.tensor_tensor(t2, t1, cb, Add)
                nc.vector.scalar_tensor_tensor(
                    dst[:, e, :], t2, g_of(e),
                    b_of(e).to_broadcast([P, TPC]), Mult, Add)

        # ================= transformer layers =================
        for l in range(nl):
            # ---- LN1 ----
            layer_norm(lambda e: lng_sb[:, l, 0, e:e + 1],
                       lambda e: lnb_sb[:, l, 0, e:e + 1], hT)

            # ---- K first, so its AllGather overlaps Q/V compute ----
            wk_t = [wp.tile([P, D], BF16, tag="wproj", bufs=12,
                            name=f"wk{l}_{e}") for e in range(EC)]
            for e in range(EC):
                nc.sync.dma_start(wk_t[e], wk.ap()[l, e * P:(e + 1) * P, :])
            for dsl in range(EC):
                psk = pp.tile([P, TPC], F32, tag="accA", bufs=4, name="psk")
                c = []
                for e in range(EC):
                    mm_chain(c, psk, wk_t[e][:, dsl * P:(dsl + 1) * P],
                             hT[:, e, :], e == 0, e == EC - 1)
                nc.vector.tensor_scalar(
                    kTl[:, dsl, :], psk, bq_sb[:, l, 1, dsl:dsl + 1], None, Add)
            agk = dram.tile([NK], BF16, tag="agin", bufs=2)
            agko = dram.tile([NCORE * NK], BF16, tag="agout", bufs=2,
                             addr_space="Shared")
            nc.sync.dma_start(
                agk.rearrange("(ec p q) -> p ec q", p=P, q=TPC), kTl)
            nc.gpsimd.collective_compute(
                "AllGather", mybir.AluOpType.bypass, replica_groups=RG,
                ins=[agk.opt()], outs=[agko.opt()])

            # ---- Q_T ----
            wq_t = [wp.tile([P, D], BF16, tag="wproj", bufs=12,
                            name=f"wq{l}_{e}") for e in range(EC)]
            for e in range(EC):
                nc.sync.dma_start(wq_t[e], wq.ap()[l, e * P:(e + 1) * P, :])
            for dsl in range(EC):
                psq = pp.tile([P, TPC], F32, tag="accA", bufs=4, name="psq")
                c = []
                for e in range(EC):
                    mm_chain(c, psq, wq_t[e][:, dsl * P:(dsl + 1) * P],
                             hT[:, e, :], e == 0, e == EC - 1)
                nc.vector.tensor_scalar(
                    qT[:, dsl, :], psq, bq_sb[:, l, 0, dsl:dsl + 1], None, Add)

            # ---- V (normal layout, with ones columns) ----
            wv_t = [wp.tile([P, D], BF16, tag="wproj", bufs=12,
                            name=f"wv{l}_{e}") for e in range(EC)]
            for e in range(EC):
                nc.sync.dma_start(wv_t[e], wv.ap()[l, e * P:(e + 1) * P, :])
            nc.sync.dma_start(vbrow, vbr.ap()[l:l + 1, :])
            nc.gpsimd.partition_broadcast(vbb, vbrow)
            for w in range(2):
                for dh in range(2):
                    psv = pp.tile([P, 384], F32, tag="accA", bufs=4,
                                  name="psv")
                    c = []
                    for e in range(EC):
                        mm_chain(c, psv, hT[:, e, w * P:(w + 1) * P],
                                 wv_t[e][:, dh * 384:(dh + 1) * 384],
                                 e == 0, e == EC - 1)
                    dst = vaug[:, w, :].rearrange(
                        "p (h x) -> p h x", h=H)[:, dh * 6:(dh + 1) * 6, 0:HD]
                    nc.vector.tensor_tensor(
                        dst, psv.rearrange("p (h x) -> p h x", h=6),
                        vbb[:, dh * 384:(dh + 1) * 384].rearrange(
                            "p (h x) -> p h x", h=6), Add)
                nc.vector.memset(
                    vaug[:, w, :].rearrange("p (h x) -> p h x", h=H)[:, :, HD:],
                    1.0)
            agv = dram.tile([NV], BF16, tag="agin", bufs=2)
            agvo = dram.tile([NCORE * NV], BF16, tag="agout", bufs=2,
                             addr_space="Shared")
            nc.sync.dma_start(
                agv.rearrange("(u p x) -> p u x", p=P, x=VAUG), vaug)
            nc.gpsimd.collective_compute(
                "AllGather", mybir.AluOpType.bypass, replica_groups=RG,
                ins=[agv.opt()], outs=[agvo.opt()])

            ktf = wp.tile([P, EC, S], BF16, tag="ktf", bufs=1)
            vfull = wp.tile([P, NB, VAUG], BF16, tag="vfull", bufs=1)
            for r in range(NCORE):
                segk = agko[r * NK:(r + 1) * NK].rearrange(
                    "(ec p q) -> ec p q", p=P, q=TPC)
                for e in range(EC):
                    nc.sync.dma_start(ktf[:, e, r * TPC:(r + 1) * TPC],
                                      segk[e])
                segv = agvo[r * NV:(r + 1) * NV].rearrange(
                    "(u p x) -> u p x", p=P, x=VAUG)
                for u in range(2):
                    nc.sync.dma_start(vfull[:, 2 * r + u, :], segv[u])

            # prefetch out-proj + fc1 weights during attention
            wo_t = [wp.tile([P, D], BF16, tag="wproj", bufs=12,
                            name=f"wo{l}_{e}") for e in range(EC)]
            for e in range(EC):
                nc.sync.dma_start(wo_t[e], wo.ap()[l, e * P:(e + 1) * P, :])
            w1_t = [wp.tile([P, FF], BF16, tag="w1", bufs=6,
                            name=f"w1{l}_{e}") for e in range(EC)]
            for e in range(EC):
                nc.sync.dma_start(w1_t[e], w1.ap()[l, e * P:(e + 1) * P, :])

            # ---- attention ----
            # sigma groups: (start, count, qlo) — qlo=0 spans both q-blocks
            groups = [(0, 2, 0), (2, 2, 0), (4, 2, 0), (6, 2, 0),
                      (8, 4, P), (12, 4, P)]
            for p_ in range(EC):  # head pairs
                pv = [pp.tile([HD + 1, TPC], F32, tag="accA", bufs=4,
                              name=f"pv{p_}_{ph}") for ph in range(2)]
                pvc = [[], []]
                for (s0, cnt, qlo) in groups:
                    qw = TPC - qlo
                    sc = [pp.tile([P, 512], F32, tag="accA", bufs=4,
                                  name=f"sc{p_}_{ph}") for ph in range(2)]
                    scc = [[], []]
                    # interleave the two heads' score MMs: their K=64
                    # row-groups differ, so LDWEIGHTS overlaps the other
                    # head's in-flight matmul.
                    for i in range(cnt):
                        sg = s0 + i
                        for ph in range(2):
                            rows = slice(ph * HD, (ph + 1) * HD)
                            mm_chain(scc[ph], sc[ph][:, i * qw:(i + 1) * qw],
                                     ktf[rows, p_, _kcol(sg):_kcol(sg) + P],
                                     qT[rows, p_, qlo:], i == 0, i == cnt - 1)
                    for ph in range(2):
                        ex = work.tile([P, 512], BF16, tag="expT", bufs=4)
                        nc.scalar.activation(ex, sc[ph], AF.Exp, scale=SCALE)
                        nc.vector.tensor_tensor(
                            ex.rearrange("p (i q) -> p i q",
                                         i=512 // qw)[:, 0:cnt, :],
                            ex.rearrange("p (i q) -> p i q",
                                         i=512 // qw)[:, 0:cnt, :],
                            msk[:, s0:s0 + cnt, qlo:], Mult)
                        h_ = 2 * p_ + ph
                        for i in range(cnt):
                            sg = s0 + i
                            mm_chain(pvc[ph],
                                     pv[ph][0:HD + 1, qlo:],
                                     vfull[:, _vchunk(sg),
                                           h_ * (HD + 1):(h_ + 1) * (HD + 1)],
                                     ex[:, i * qw:(i + 1) * qw],
                                     sg == 0, sg == NB - 1)
                # normalize + write attention output (transposed layout)
                for ph in range(2):
                    rs = work.tile([1, TPC], F32, tag="rs")
                    nc.vector.tensor_copy(rs, pv[ph][HD:HD + 1, :])
                    nc.vector.reciprocal_approx_fast(rs, rs)
                    rb = work.tile([HD, TPC], F32, tag="rb")
                    nc.gpsimd.partition_broadcast(rb, rs)
                    nc.vector.tensor_tensor(
                        xaT[ph * HD:(ph + 1) * HD, p_, :],
                        pv[ph][0:HD, :], rb, Mult)

            # ---- out projection + residual ----
            for e in range(EC):
                pso = pp.tile([P, TPC], F32, tag="accA", bufs=4, name="pso")
                c = []
                for f in range(EC):
                    mm_chain(c, pso, wo_t[f][:, e * P:(e + 1) * P],
                             xaT[:, f, :], f == 0, f == EC - 1)
                nc.vector.scalar_tensor_tensor(
                    xT[:, e, :], pso, bq_sb[:, l, 3, e:e + 1], xT[:, e, :],
                    Add, Add)

            # ---- LN2 + MLP ----
            layer_norm(lambda e: lng_sb[:, l, 1, e:e + 1],
                       lambda e: lnb_sb[:, l, 1, e:e + 1], hT)
            for f in range(FS):
                ps1 = pp.tile([P, TPC], F32, tag="accA", bufs=4, name="ps1")
                c = []
                for e in range(EC):
                    mm_chain(c, ps1, w1_t[e][:, f * P:(f + 1) * P],
                             hT[:, e, :], e == 0, e == EC - 1)
                nc.scalar.activation(gT[:, f, :], ps1, AF.Gelu,
                                     bias=b1_sb[:, l, f:f + 1])
            # fc2: f-outer so w2 tiles stream; one 3-bank accumulator
            ps2 = pp.tile([P, EC, TPC], F32, tag="accB", bufs=1, name="ps2")
            c2ch = []
            for f in range(FS):
                w2t = wp.tile([P, D], BF16, tag="w2", bufs=4,
                              name=f"w2{l}_{f}")
                nc.sync.dma_start(w2t, w2.ap()[l, f * P:(f + 1) * P, :])
                for e in range(EC):
                    mm_chain(c2ch, ps2[:, e, :],
                             w2t[:, e * P:(e + 1) * P], gT[:, f, :],
                             f == 0 and e % 2 == 0, f == FS - 1)
            for e in range(EC):
                nc.vector.scalar_tensor_tensor(
                    xT[:, e, :], ps2[:, e, :], b2_sb[:, l, e:e + 1],
                    xT[:, e, :], Add, Add)

        # ================= final LN + lm_head =================
        layer_norm(lambda e: lnf_sb[:, 0, e:e + 1],
                   lambda e: lnf_sb[:, 1, e:e + 1], hT)

        zin = dram.tile([NZ], BF16, tag="agin", bufs=2)
        zout = dram.tile([NCORE * NZ], BF16, tag="agout", bufs=2,
                         addr_space="Shared")
        nc.sync.dma_start(
            zin.rearrange("(ec p q) -> p ec q", p=P, q=TPC), hT)
        nc.gpsimd.collective_compute(
            "AllGather", mybir.AluOpType.bypass, replica_groups=RG,
            ins=[zin.opt()], outs=[zout.opt()])
        zfull = wp.tile([P, EC, S], BF16, tag="ktf", bufs=1)
        for r in range(NCORE):
            segz = zout[r * NZ:(r + 1) * NZ].rearrange(
                "(ec p q) -> ec p q", p=P, q=TPC)
            for e in range(EC):
                nc.sync.dma_start(zfull[:, e, r * TPC:(r + 1) * TPC], segz[e])

        # lm_head: group 4 v-chunks per wlm load; keep the z t-chunk stationary
        # across the 4 chunks so walrus dedupes 3 of every 4 LDWEIGHTS.
        vgroups = []
        i = 0
        while i * 512 < VS:
            g = []
            for _ in range(2):
                if i * 512 >= VS:
                    break
                g.append((i * 512, min(512, VS - i * 512)))
                i += 1
            vgroups.append(g)
        for g in vgroups:
            wlm_t = wp.tile([P, EC, 2 * 512], BF16, tag="wlm", bufs=2,
                            name="wlmt")
            for e in range(EC):
                for j, (v0, vw) in enumerate(g):
                    nc.sync.dma_start(
                        wlm_t[:, e, j * 512:j * 512 + vw],
                        wlm.ap()[e * P:(e + 1) * P, v0:v0 + vw])
            for t in range(NB):
                pslm = [pp.tile([P, 512], F32, tag="accA", bufs=4,
                                name=f"pslm{j}") for j in range(len(g))]
                allc = []
                for e in range(EC):
                    for j, (v0, vw) in enumerate(g):
                        mm_chain(allc, pslm[j][:, 0:vw],
                                 zfull[:, e, t * P:(t + 1) * P],
                                 wlm_t[:, e, j * 512:j * 512 + vw],
                                 e == 0, e == EC - 1)
                b = _block_of_tchunk(t)
                for j, (v0, vw) in enumerate(g):
                    ob = work.tile([P, 512], F32, tag="lmout", bufs=4)
                    nc.any.tensor_copy(ob[:, 0:vw], pslm[j][:, 0:vw])
                    nc.sync.dma_start(
                        out.ap()[b * P:(b + 1) * P, v0:v0 + vw], ob[:, 0:vw])

    nc.compile()
    return nc


# ======================= host side =======================

def shard_inputs(input_ids, params, nl=L):
    """Build the 8 per-core input maps from the full model inputs."""
    ids = np.asarray(input_ids).reshape(-1)
    p = params
    blocks = p['blocks']
    f32 = np.float32

    def _np(x):
        return np.asarray(x)

    def _pm3(a):
        # [..., D] -> partition-major [128, ..., D//128]
        lead = a.shape[:-1]
        a = a.reshape(*lead, -1, P)
        perm = (a.ndim - 1,) + tuple(range(a.ndim - 1))
        return np.ascontiguousarray(a.transpose(perm)).astype(f32)

    _pm4 = _pm3

    com = {
        'wq': _np(blocks['q_w'])[:nl].astype(BF),
        'wk': _np(blocks['k_w'])[:nl].astype(BF),
        'wv': _np(blocks['v_w'])[:nl].astype(BF),
        'wo': _np(blocks['o_w'])[:nl].astype(BF),
        'w1': _np(blocks['fc1_w'])[:nl].astype(BF),
        'w2': _np(blocks['fc2_w'])[:nl].astype(BF),
        'bqkv': _pm4(np.stack([_np(blocks['q_b']), _np(blocks['k_b']),
                               _np(blocks['v_b']), _np(blocks['o_b'])],
                              axis=1)[:nl]),
        'b1d': _pm3(_np(blocks['fc1_b'])[:nl]),
        'b2d': _pm3(_np(blocks['fc2_b'])[:nl]),
        'lng': _pm4(np.stack([_np(blocks['ln1_g']), _np(blocks['ln2_g'])],
                             axis=1)[:nl]),
        'lnb': _pm4(np.stack([_np(blocks['ln1_b']), _np(blocks['ln2_b'])],
                             axis=1)[:nl]),
        'lnf': _pm3(np.stack([_np(p['ln_f_g']), _np(p['ln_f_b'])])),
        'vbr': _np(blocks['v_b'])[:nl].astype(f32),
    }
    wte = _np(p['wte'])
    wpe = _np(p['wpe'])
    lm = _np(p['lm_head'])

    in_maps = []
    for c in range(NCORE):
        b0, b1 = c, 15 - c
        tok = np.concatenate([ids[b0 * P:(b0 + 1) * P],
                              ids[b1 * P:(b1 + 1) * P]])
        pos = np.concatenate([np.arange(b0 * P, (b0 + 1) * P),
                              np.arange(b1 * P, (b1 + 1) * P)])
        xw = np.ascontiguousarray(
            wte[tok].T.reshape(EC, P, TPC)).astype(f32)
        xp = np.ascontiguousarray(
            wpe[pos].T.reshape(EC, P, TPC)).astype(f32)
        # mask[k, sigma, w*128+i] for scores_T: keep iff global_k <= global_q
        m = np.zeros((P, NB, TPC), np.float32)
        tri = np.triu(np.ones((P, P), np.float32))  # [k, q]: keep q >= k
        for sg in range(NB):
            for w, bq in ((0, b0), (1, b1)):
                if sg < bq:
                    m[:, sg, w * P:(w + 1) * P] = 1.0
                elif sg == bq:
                    m[:, sg, w * P:(w + 1) * P] = tri
        in_maps.append({
            'xwte': xw, 'xwpe': xp,
            'mskd': m.astype(BF),
            'wlm': np.ascontiguousarray(lm[:, c * VS:(c + 1) * VS]).astype(BF),
            **com,
        })
    return in_maps


def assemble_output(results):
    parts = [np.asarray(r['logits']) for r in results]
    return np.concatenate(parts, axis=1)[None].astype(np.float32)


_NC_CACHE = {}


def get_nc(nl=L):
    if nl not in _NC_CACHE:
        _NC_CACHE[nl] = build_nc(nl)
    return _NC_CACHE[nl]


def kernel(input_ids, params, _trace=False):
    from concourse.bass_utils import run_bass_kernel_spmd
    nc = get_nc()
    in_maps = shard_inputs(input_ids, params)
    res = run_bass_kernel_spmd(nc, in_maps, core_ids=list(range(NCORE)),
                               trace=_trace)
    out = assemble_output(res.results)
    if _trace:
        return out, res
    return out
